# Initial kernel scaffold
#
"""Row-wise cosine similarity kernel for Trainium2 (Bass/Tile).

Computes out[b, n] = cos(a[b, n, :], b[b, n, :]) for a, b of shape
(16, 4096, 256) f32, distributed data-parallel across 8 NeuronCores.

Per core: 8192 rows of 256. Row r = p*64 + t lives in partition p,
group t (t = g*8 + s; 8 groups per 1 MiB DMA tile). Per 128-row group:
  DVE : tensor_tensor_reduce  -> dot = sum(a*b), sb = sum(b*b)
  ACT : activation(Square)    -> sa = sum(a*a)
Epilogue (batched over all 64 groups): out = dot / sqrt(sa*sb).
"""

import sys

for _p in ("/opt/trn_rl_repo",):
    if _p not in sys.path:
        sys.path.insert(0, _p)

import numpy as np

import concourse.bass as bass
import concourse.mybir as mybir
import concourse.tile as tile
from concourse.bass_utils import run_bass_kernel_spmd

B, N, D = 16, 4096, 256
NCORES = 8
ROWS = B * N                 # 65536
RPC = ROWS // NCORES         # 8192 rows per core
P = 128                      # partitions
GROUPS = RPC // P            # 64 groups of 128 rows per core
SPT = 8                      # groups per DMA tile (1 MiB per tensor)
TILES = GROUPS // SPT        # 8 DMA tiles per tensor per core

_cached_nc = None


def build_nc():
    nc = bass.Bass("TRN2")
    a = nc.dram_tensor("a", [RPC, D], mybir.dt.float32, kind="ExternalInput")
    b = nc.dram_tensor("b", [RPC, D], mybir.dt.float32, kind="ExternalInput")
    o = nc.dram_tensor("out", [RPC], mybir.dt.float32, kind="ExternalOutput")

    # row index = p*GROUPS + g*SPT + s; per partition each tile holds
    # SPT consecutive rows -> 8 KiB contiguous per partition per DMA.
    av = a[:, :].rearrange("(p g s) d -> g p s d", p=P, g=TILES, s=SPT)
    bv = b[:, :].rearrange("(p g s) d -> g p s d", p=P, g=TILES, s=SPT)
    ov = o[:].rearrange("(p t) -> p t", p=P)

    f32 = mybir.dt.float32
    with tile.TileContext(nc) as tc:
        with (
            tc.tile_pool(name="loads", bufs=4) as loads,
            tc.tile_pool(name="scratch", bufs=2) as scratch,
            tc.tile_pool(name="acc", bufs=1) as acc,
        ):
            sa = acc.tile([P, GROUPS], f32, tag="sa")
            sb = acc.tile([P, GROUPS], f32, tag="sb")
            dot = acc.tile([P, GROUPS], f32, tag="dot")

            for g in range(TILES):
                at = loads.tile([P, SPT, D], f32, tag="a")
                bt = loads.tile([P, SPT, D], f32, tag="b")
                nc.sync.dma_start(out=at[:, :, :], in_=av[g])
                nc.sync.dma_start(out=bt[:, :, :], in_=bv[g])
                for s in range(SPT):
                    t = g * SPT + s
                    scr_a = scratch.tile([P, D], f32, tag="scr_a")
                    scr_d = scratch.tile([P, D], f32, tag="scr_d")
                    scr_b = scratch.tile([P, D], f32, tag="scr_b")
                    nc.scalar.activation(
                        out=scr_a[:, :],
                        in_=at[:, s, :],
                        func=mybir.ActivationFunctionType.Square,
                        accum_out=sa[:, t : t + 1],
                    )
                    nc.vector.tensor_tensor_reduce(
                        out=scr_d[:, :],
                        in0=at[:, s, :],
                        in1=bt[:, s, :],
                        scale=1.0,
                        scalar=0.0,
                        op0=mybir.AluOpType.mult,
                        op1=mybir.AluOpType.add,
                        accum_out=dot[:, t : t + 1],
                    )
                    nc.vector.tensor_tensor_reduce(
                        out=scr_b[:, :],
                        in0=bt[:, s, :],
                        in1=bt[:, s, :],
                        scale=1.0,
                        scalar=0.0,
                        op0=mybir.AluOpType.mult,
                        op1=mybir.AluOpType.add,
                        accum_out=sb[:, t : t + 1],
                    )

            prod = acc.tile([P, GROUPS], f32, tag="prod")
            rs = acc.tile([P, GROUPS], f32, tag="rs")
            res = acc.tile([P, GROUPS], f32, tag="res")
            nc.vector.tensor_mul(prod[:, :], sa[:, :], sb[:, :])
            nc.scalar.activation(
                out=rs[:, :],
                in_=prod[:, :],
                func=mybir.ActivationFunctionType.Sqrt,
            )
            nc.vector.reciprocal(out=prod[:, :], in_=rs[:, :])
            nc.vector.tensor_mul(res[:, :], dot[:, :], prod[:, :])
            nc.sync.dma_start(out=ov, in_=res[:, :])
    return nc


def _get_nc():
    global _cached_nc
    if _cached_nc is None:
        _cached_nc = build_nc()
    return _cached_nc


def run(inputs, **kwargs):
    """Shard, run on 8 cores, gather. Returns (output, BassKernelResults)."""
    a = np.ascontiguousarray(np.asarray(inputs["a"], dtype=np.float32)).reshape(
        ROWS, D
    )
    b = np.ascontiguousarray(np.asarray(inputs["b"], dtype=np.float32)).reshape(
        ROWS, D
    )
    in_maps = [
        {
            "a": a[c * RPC : (c + 1) * RPC],
            "b": b[c * RPC : (c + 1) * RPC],
        }
        for c in range(NCORES)
    ]
    r = run_bass_kernel_spmd(_get_nc(), in_maps, core_ids=list(range(NCORES)), **kwargs)
    out = np.concatenate([r.results[c]["out"] for c in range(NCORES)])
    return out.reshape(B, N).astype(np.float32), r


def kernel(**inputs) -> np.ndarray:
    out, _ = run(inputs)
    return out


# revision 8
# speedup vs baseline: 1.9688x; 1.9688x over previous
"""Row-wise cosine similarity kernel for Trainium2 (Bass/Tile).

Computes out[b, n] = cos(a[b, n, :], b[b, n, :]) for a, b of shape
(16, 4096, 256) f32, distributed data-parallel across 8 NeuronCores.

Per core: 8192 rows of 256. Row r = p*64 + t lives in partition p,
group t (t = g*8 + s; 8 groups per 1 MiB DMA tile). Per 128-row group:
  DVE : tensor_tensor_reduce  -> dot = sum(a*b), sb = sum(b*b)
  ACT : activation(Square)    -> sa = sum(a*a)
Epilogue (batched over all 64 groups): out = dot / sqrt(sa*sb).
"""

import sys

for _p in ("/opt/trn_rl_repo",):
    if _p not in sys.path:
        sys.path.insert(0, _p)

import numpy as np

import concourse.bacc as bacc
import concourse.mybir as mybir
import concourse.tile as tile
from concourse.bass_utils import run_bass_kernel_spmd

B, N, D = 16, 4096, 256
NCORES = 8
ROWS = B * N                 # 65536
RPC = ROWS // NCORES         # 8192 rows per core
P = 128                      # partitions
GROUPS = RPC // P            # 64 groups of 128 rows per core
SPT = 8                      # groups per DMA tile (1 MiB per tensor)
TILES = GROUPS // SPT        # 8 DMA tiles per tensor per core

_cached_nc = None


def build_nc(reps=1, spt=SPT, load_bufs=4, scratch_bufs=2):
    tiles = GROUPS // spt
    nc = bacc.Bacc("TRN2", target_bir_lowering=False)
    a = nc.dram_tensor("a", [RPC, D], mybir.dt.float32, kind="ExternalInput")
    b = nc.dram_tensor("b", [RPC, D], mybir.dt.float32, kind="ExternalInput")
    o = nc.dram_tensor("out", [RPC], mybir.dt.float32, kind="ExternalOutput")

    # row index = p*GROUPS + g*spt + s; per partition each tile holds
    # spt consecutive rows -> spt KiB contiguous per partition per DMA.
    av = a[:, :].rearrange("(p g s) d -> g p s d", p=P, g=tiles, s=spt)
    bv = b[:, :].rearrange("(p g s) d -> g p s d", p=P, g=tiles, s=spt)
    ov = o[:].rearrange("(p t) -> p t", p=P)

    with tile.TileContext(nc) as tc:
        with (
            tc.tile_pool(name="loads", bufs=load_bufs) as loads,
            tc.tile_pool(name="scratch", bufs=scratch_bufs) as scratch,
            tc.tile_pool(name="acc", bufs=1) as acc,
        ):
            for _rep in range(reps):
                _body(nc, loads, scratch, acc, av, bv, ov, tiles, spt)
    nc.compile()
    return nc


def _body(nc, loads, scratch, acc, av, bv, ov, tiles, spt):
    f32 = mybir.dt.float32
    if True:
        if True:
            sa = acc.tile([P, GROUPS], f32, tag="sa")
            sb = acc.tile([P, GROUPS], f32, tag="sb")
            dot = acc.tile([P, GROUPS], f32, tag="dot")

            for g in range(tiles):
                at = loads.tile([P, spt, D], f32, tag="a")
                bt = loads.tile([P, spt, D], f32, tag="b")
                nc.sync.dma_start(out=at[:, :, :], in_=av[g])
                nc.sync.dma_start(out=bt[:, :, :], in_=bv[g])
                for s in range(spt):
                    t = g * spt + s
                    scr_a = scratch.tile([P, D], f32, tag="scr_a")
                    scr_d = scratch.tile([P, D], f32, tag="scr_d")
                    scr_b = scratch.tile([P, D], f32, tag="scr_b")
                    nc.scalar.activation(
                        out=scr_a[:, :],
                        in_=at[:, s, :],
                        func=mybir.ActivationFunctionType.Square,
                        accum_out=sa[:, t : t + 1],
                    )
                    nc.vector.affine_mul_reduce(
                        out=scr_d[:, :],
                        accum_out=dot[:, t : t + 1],
                        in0=at[:, s, :],
                        in1=bt[:, s, :],
                        scale=1.0,
                        bias=0.0,
                    )
                    nc.vector.affine_mul_reduce(
                        out=scr_b[:, :],
                        accum_out=sb[:, t : t + 1],
                        in0=bt[:, s, :],
                        in1=bt[:, s, :],
                        scale=1.0,
                        bias=0.0,
                    )

            prod = acc.tile([P, GROUPS], f32, tag="prod")
            rs = acc.tile([P, GROUPS], f32, tag="rs")
            res = acc.tile([P, GROUPS], f32, tag="res")
            nc.vector.tensor_mul(prod[:, :], sa[:, :], sb[:, :])
            nc.scalar.activation(
                out=rs[:, :],
                in_=prod[:, :],
                func=mybir.ActivationFunctionType.Sqrt,
            )
            nc.vector.reciprocal(out=prod[:, :], in_=rs[:, :])
            nc.vector.tensor_mul(res[:, :], dot[:, :], prod[:, :])
            nc.sync.dma_start(out=ov, in_=res[:, :])


def _get_nc():
    global _cached_nc
    if _cached_nc is None:
        _cached_nc = build_nc()
    return _cached_nc


def run(inputs, **kwargs):
    """Shard, run on 8 cores, gather. Returns (output, BassKernelResults)."""
    a = np.ascontiguousarray(np.asarray(inputs["a"], dtype=np.float32)).reshape(
        ROWS, D
    )
    b = np.ascontiguousarray(np.asarray(inputs["b"], dtype=np.float32)).reshape(
        ROWS, D
    )
    in_maps = [
        {
            "a": a[c * RPC : (c + 1) * RPC],
            "b": b[c * RPC : (c + 1) * RPC],
        }
        for c in range(NCORES)
    ]
    r = run_bass_kernel_spmd(_get_nc(), in_maps, core_ids=list(range(NCORES)), **kwargs)
    out = np.concatenate([r.results[c]["out"] for c in range(NCORES)])
    return out.reshape(B, N).astype(np.float32), r


def kernel(**inputs) -> np.ndarray:
    out, _ = run(inputs)
    return out
